# revision 9
# baseline (speedup 1.0000x reference)
"""GAT layer (nn_ManualGATLayer) Bass/Tile kernel for 8 Trainium2 cores.

Math (per head h, batch b):
    Wh   = h_b @ W_h.T                          [N, D]
    si   = Wh @ a1,  sj = Wh @ a2               [N]
    e_ij = leaky(si_i + sj_j), masked by adj, softmax over j, out = alpha @ Wh

Key identities:
  1) leaky(x) = max(x, 0.2x) and exp is monotone, so
         exp(leaky(si_i + sj_j)) = max(Ei*Ej, Fi*Fj)
     with Ei = exp(si), Ej = exp(sj), Fi = exp(0.2 si), Fj = exp(0.2 sj).
  2) Softmax over j is invariant to any per-i row scale, so we compute
         P'_ij = P_ij / Ei = max(Hi_i * Fj_j, Ej_j),   Hi = exp(-0.8 si)
     In the [j(128-part) x i(free)] tile layout, Hi is the broadcast
     tensor and Ej/Fj are per-partition scalars, so the WHOLE N^2 score
     needs ONE two-op tensor_scalar:  (Hi_bc mult Fj[p]) max Ej[p]
     running in 4x bf16 mode (~594ns/tile vs 2315ns for the 3-op split).

The adjacency mask is folded into the DMA load: adjT' = {0, -57344}
stored fp8e5m2 is added onto the score tile via SWDGE accum_op=add;
-57344 swamps any score (<~100) and a relu restores exact zeros.

Consume is "flipped": the P' tile is the matmul STATIONARY operand
(16 blocks of [128j x 128i] per tile) and the moving operand is
wh_ext = [Wh_tile | 4.0] (129 cols), so psum[i, 0:128] accumulates
out-numerator^T' and psum[i, 128] accumulates 4*r'_i: the rowsum rides
along for +1 column instead of a second full matmul pass. The host
divides by column 128 (softmax denominator; the 4.0 folds the H-head
mean), which also cancels the 1/Ei row scale exactly.

Sharding: one (h, b) pair per core (H*B = 8 = n_cores); host sums heads.
"""

import numpy as np
import ml_dtypes

BF16 = ml_dtypes.bfloat16
NEG_SLOPE = 0.2
MASK_VAL = -57344.0

# Problem sizes (hardcoded per contest contract).
B, N, IN, D, H = 2, 4096, 256, 128, 4
N_CORES = 8

_cache = {}


def _build(n=N, n_in=IN, d=D, num_devices=N_CORES, repeat=1, skip=(),
           relu_act_every=4, score_2op=True):
    # skip: subset of {"dve", "dma", "relu", "mm"} for timing-attribution
    # variants (numerically wrong where used).
    # relu_act_every: every k-th tile's relu runs on ACT instead of DVE
    # (0 = never, 1 = always).
    import concourse.bacc as bacc
    import concourse.tile as tile
    from concourse import mybir

    f32 = mybir.dt.float32
    bf16 = mybir.dt.bfloat16
    AF = mybir.ActivationFunctionType
    ALU = mybir.AluOpType

    n_jt = n // 128          # j tiles of 128
    ih_n = 2                 # i halves
    iw = n // ih_n           # i width per half (2048)
    n_ib = iw // 128         # i blocks per half (16) -- two per PSUM bank
    n_kt = n_in // 128       # contraction tiles for Wh

    nc = bacc.Bacc(
        "TRN2",
        target_bir_lowering=False,
        debug=False,
        num_devices=num_devices,
    )

    hb = nc.dram_tensor("hb", [n, n_in], bf16, kind="ExternalInput")
    wt = nc.dram_tensor("wt", [n_in, d], bf16, kind="ExternalInput")
    a12 = nc.dram_tensor("a12", [d, 2], bf16, kind="ExternalInput")
    adjt = nc.dram_tensor("adjt", [n, n], mybir.dt.float8e5,
                          kind="ExternalInput")
    # out rows: [i, 0:128] = unnormalized out^T', [i, 128] = 4*r'_i
    outR = nc.dram_tensor("outR", [n, d + 1], f32, kind="ExternalOutput")

    with tile.TileContext(nc) as tc:
        with tc.tile_pool(name="const", bufs=1) as const:
            # --- constants and persistent tiles ---
            wt_sb = const.tile([128, n_kt, d], bf16)
            nc.sync.dma_start(
                out=wt_sb, in_=wt[:].rearrange("(k p) d -> p k d", p=128)
            )
            a12_sb = const.tile([d, 2], bf16)
            nc.sync.dma_start(out=a12_sb, in_=a12[:])
            ones1 = const.tile([1, 128], f32)
            nc.vector.memset(ones1, 1.0)

            # hT[k] = h[:, k*128:(k+1)*128].T  via DMA xbar transpose
            hT = const.tile([128, n_kt, n], bf16)
            for k in range(n_kt):
                nc.sync.dma_start(
                    out=hT[:, k, :],
                    in_=hb[:, k * 128 : (k + 1) * 128],
                    transpose=True,
                )

            # --- wh_ext tiles [n-tile, 129] (bf16): [Wh | 4.0] ---
            wh_ext = const.tile([128, n_jt, d + 1], bf16)
            nc.vector.memset(wh_ext, float(H))  # col d = 4.0 (head mean)
            with tc.tile_pool(name="ps_wh", bufs=2, space="PSUM") as ps_wh:
                for g in range(n_jt // 4):
                    wh_ps = ps_wh.tile([128, 4, d], f32, tag="wh_ps")
                    for q in range(4):
                        nt = g * 4 + q
                        for k in range(n_kt):
                            nc.tensor.matmul(
                                wh_ps[:, q, :],
                                hT[:, k, nt * 128 : (nt + 1) * 128],
                                wt_sb[:, k, :],
                                start=(k == 0),
                                stop=(k == n_kt - 1),
                            )
                    for q in range(4):
                        nc.vector.tensor_copy(
                            wh_ext[:, g * 4 + q, 0:d], wh_ps[:, q, :]
                        )

            # --- WhT [d, n] (bf16) ---
            whT_sb = const.tile([128, n], bf16)
            with tc.tile_pool(name="ps_whT", bufs=1, space="PSUM") as ps_whT:
                whT_ps = ps_whT.tile([128, n], f32)
                for c in range(n // 512):
                    for k in range(n_kt):
                        nc.tensor.matmul(
                            whT_ps[:, c * 512 : (c + 1) * 512],
                            wt_sb[:, k, :],
                            hT[:, k, c * 512 : (c + 1) * 512],
                            start=(k == 0),
                            stop=(k == n_kt - 1),
                        )
                nc.vector.tensor_copy(whT_sb, whT_ps)

            # --- si/sj row vectors [2, n] f32 ---
            s_sb = const.tile([2, n], f32)
            with tc.tile_pool(name="ps_s", bufs=1, space="PSUM") as ps_s:
                s_ps = ps_s.tile([2, n], f32)
                for c in range(n // 512):
                    nc.tensor.matmul(
                        s_ps[:, c * 512 : (c + 1) * 512],
                        a12_sb,
                        whT_sb[:, c * 512 : (c + 1) * 512],
                        start=True,
                        stop=True,
                    )
                nc.vector.tensor_copy(s_sb, s_ps)

            # --- Hi broadcast tile [128, n] bf16: exp(-0.8 si) ---
            Hi_bc = const.tile([128, n], bf16)
            with tc.tile_pool(name="ps_sib", bufs=1, space="PSUM") as ps_sib:
                sib_ps = ps_sib.tile([128, n], f32)
                for c in range(n // 512):
                    nc.tensor.matmul(
                        sib_ps[:, c * 512 : (c + 1) * 512],
                        ones1,
                        s_sb[0:1, c * 512 : (c + 1) * 512],
                        start=True,
                        stop=True,
                    )
                nc.scalar.activation(Hi_bc, sib_ps, AF.Exp, scale=-0.8)

            # --- Ej/Fj per-partition columns [128, n_jt] f32 ---
            Ej_cols = const.tile([128, n_jt], f32)
            Fj_cols = const.tile([128, n_jt], f32)
            with tc.tile_pool(name="ps_sj", bufs=1, space="PSUM") as ps_sj:
                sj_ps = ps_sj.tile([128, n_jt], f32)
                for t in range(n_jt):
                    nc.tensor.matmul(
                        sj_ps[:, t : t + 1],
                        whT_sb[:, t * 128 : (t + 1) * 128],
                        a12_sb[:, 1:2],
                        start=True,
                        stop=True,
                    )
                nc.scalar.activation(Ej_cols, sj_ps, AF.Exp)
                nc.scalar.activation(Fj_cols, sj_ps, AF.Exp, scale=NEG_SLOPE)

            # --- main attention loop ---
            with (
                tc.tile_pool(name="work", bufs=4) as work,
                tc.tile_pool(name="fin", bufs=2) as fin,
                tc.tile_pool(name="ps_main", bufs=1, space="PSUM") as ps_main,
            ):
                for ih in [x for x in range(ih_n)] * repeat:
                    i0 = ih * iw
                    # 16 psum blocks of [128 i, 129(pad 256)] f32, two per
                    # 2KB bank.  start=True clears has_written for the WHOLE
                    # bank, so only the even block of each bank pair issues
                    # it; the odd block's first matmul relies on
                    # "overwrite where has_written is unset" (its bits were
                    # cleared by the even neighbor's start, which the issue
                    # order guarantees happens first).
                    out_ps = ps_main.tile([128, n_ib, 256], f32, tag="out_ps")
                    for jt in range(n_jt):
                        m = work.tile([128, iw], bf16, tag="m")
                        if "dve" not in skip:
                            # P'/relu pre-mask: max(Hi_i * Fj_j, Ej_j)
                            if score_2op:
                                nc.vector.tensor_scalar(
                                    m,
                                    Hi_bc[:, i0 : i0 + iw],
                                    Fj_cols[:, jt : jt + 1],
                                    Ej_cols[:, jt : jt + 1],
                                    ALU.mult,
                                    ALU.max,
                                )
                            else:
                                t = work.tile([128, iw], bf16, tag="t")
                                nc.vector.tensor_scalar_mul(
                                    t,
                                    Hi_bc[:, i0 : i0 + iw],
                                    Fj_cols[:, jt : jt + 1],
                                )
                                nc.vector.tensor_scalar(
                                    m,
                                    t,
                                    Ej_cols[:, jt : jt + 1],
                                    None,
                                    ALU.max,
                                )
                        else:
                            nc.vector.memset(m, 1.0)
                        if "dma" not in skip:
                            # fold adjacency mask in during the load:
                            #   m += adjT' ({0, -57344}), then P' = relu(m).
                            nc.gpsimd.dma_start(
                                out=m,
                                in_=adjt[jt * 128 : (jt + 1) * 128,
                                         i0 : i0 + iw],
                                accum_op=ALU.add,
                            )
                        if "relu" not in skip:
                            p = work.tile([128, iw], bf16, tag="p")
                            if relu_act_every and jt % relu_act_every == 0:
                                nc.scalar.activation(p, m, AF.Relu)
                            else:
                                nc.vector.tensor_scalar_max(p, m, 0.0)
                        else:
                            p = m
                        if "mm" in skip:
                            continue
                        for bi in range(n_ib):
                            nc.tensor.matmul(
                                out_ps[:, bi, 0 : d + 1],
                                p[:, bi * 128 : (bi + 1) * 128],
                                wh_ext[:, jt, :],
                                start=(jt == 0 and bi % 2 == 0),
                                stop=(jt == n_jt - 1),
                                skip_group_check=(bi % 2 == 1),
                            )
                    # drain: one ACT copy for all 16 blocks, then DMA out
                    out_sb = fin.tile([128, n_ib, d + 1], f32, tag="out_sb")
                    if "mm" in skip:
                        nc.vector.memset(out_sb, 1.0)
                    else:
                        nc.scalar.activation(
                            out_sb, out_ps[:, :, 0 : d + 1], AF.Copy
                        )
                    nc.sync.dma_start(
                        out=outR[i0 : i0 + iw, :].rearrange(
                            "(b p) c -> p b c", p=128
                        ),
                        in_=out_sb,
                    )

    nc.compile()
    return nc


def _prep_inputs(h, adj, W, a):
    """Host-side shard/layout prep. Returns list of 8 per-core input dicts."""
    h_bf = np.asarray(h).astype(BF16)
    adjt_big = np.where(np.asarray(adj).T != 0, 0.0, MASK_VAL).astype(
        ml_dtypes.float8_e5m2
    )
    adjt_big = np.ascontiguousarray(adjt_big)
    W = np.asarray(W)
    a = np.asarray(a)
    in_maps = []
    for c in range(N_CORES):
        hd, b = divmod(c, B)
        wt = np.ascontiguousarray(W[hd].T).astype(BF16)          # [IN, D]
        a12 = np.stack([a[hd, :D], a[hd, D:]], axis=1).astype(BF16)  # [D, 2]
        in_maps.append(
            {"hb": np.ascontiguousarray(h_bf[b]), "wt": wt, "a12": a12,
             "adjt": adjt_big}
        )
    return in_maps


def kernel(h, adj, W, a):
    from concourse.bass_utils import run_bass_kernel_spmd

    if "nc" not in _cache:
        _cache["nc"] = _build()
    nc = _cache["nc"]

    in_maps = _prep_inputs(h, adj, W, a)
    res = run_bass_kernel_spmd(nc, in_maps, core_ids=list(range(N_CORES)))
    outs = [r["outR"] for r in res.results]  # each [N, D+1] f32

    out = np.zeros((B, N, D), dtype=np.float32)
    for c in range(N_CORES):
        hd, b = divmod(c, B)
        o = outs[c]
        r = o[:, D:]
        out[b] += np.divide(o[:, :D], r, out=np.zeros((N, D), np.float32),
                            where=r != 0)
    return out


# revision 10
# speedup vs baseline: 1.1429x; 1.1429x over previous
"""GAT layer (nn_ManualGATLayer) Bass/Tile kernel for 8 Trainium2 cores.

Math (per head h, batch b):
    Wh   = h_b @ W_h.T                          [N, D]
    si   = Wh @ a1,  sj = Wh @ a2               [N]
    e_ij = leaky(si_i + sj_j), masked by adj, softmax over j, out = alpha @ Wh

Key identities:
  1) leaky(x) = max(x, 0.2x) and exp is monotone, so
         exp(leaky(si_i + sj_j)) = max(Ei*Ej, Fi*Fj)
     with Ei = exp(si), Ej = exp(sj), Fi = exp(0.2 si), Fj = exp(0.2 sj).
  2) Softmax over j is invariant to any per-i row scale, so we compute
         P'_ij = P_ij / Ei = max(Hi_i * Fj_j, Ej_j),   Hi = exp(-0.8 si)
     In the [j(128-part) x i(free)] tile layout, Hi is the broadcast
     tensor and Ej/Fj are per-partition scalars, so the WHOLE N^2 score
     needs ONE two-op tensor_scalar:  (Hi_bc mult Fj[p]) max Ej[p]
     running in 4x bf16 mode (~594ns/tile vs 2315ns for the 3-op split).

The adjacency mask is folded into the DMA load: adjT' = {0, -57344}
stored fp8e5m2 is added onto the score tile via SWDGE accum_op=add;
-57344 swamps any score (<~100) and a relu restores exact zeros.

Consume is "flipped": the P' tile is the matmul STATIONARY operand
(16 blocks of [128j x 128i] per tile) and the moving operand is
wh_ext = [Wh_tile | 4.0] (129 cols), so psum[i, 0:128] accumulates
out-numerator^T' and psum[i, 128] accumulates 4*r'_i: the rowsum rides
along for +1 column instead of a second full matmul pass. The host
divides by column 128 (softmax denominator; the 4.0 folds the H-head
mean), which also cancels the 1/Ei row scale exactly.

Sharding: one (h, b) pair per core (H*B = 8 = n_cores); host sums heads.
"""

import numpy as np
import ml_dtypes

BF16 = ml_dtypes.bfloat16
NEG_SLOPE = 0.2
MASK_VAL = -57344.0

# Problem sizes (hardcoded per contest contract).
B, N, IN, D, H = 2, 4096, 256, 128, 4
N_CORES = 8

_cache = {}


def _build(n=N, n_in=IN, d=D, num_devices=N_CORES, repeat=1, skip=(),
           relu_act_every=4, score_2op=True):
    # skip: subset of {"dve", "dma", "relu", "mm"} for timing-attribution
    # variants (numerically wrong where used).
    # relu_act_every: every k-th tile's relu runs on ACT instead of DVE
    # (0 = never, 1 = always).
    import concourse.bacc as bacc
    import concourse.tile as tile
    from concourse import mybir

    f32 = mybir.dt.float32
    bf16 = mybir.dt.bfloat16
    AF = mybir.ActivationFunctionType
    ALU = mybir.AluOpType

    n_jt = n // 128          # j tiles of 128
    ih_n = 2                 # i halves
    iw = n // ih_n           # i width per half (2048)
    n_ib = iw // 128         # i blocks per half (16) -- two per PSUM bank
    n_kt = n_in // 128       # contraction tiles for Wh

    nc = bacc.Bacc(
        "TRN2",
        target_bir_lowering=False,
        debug=False,
        num_devices=num_devices,
    )

    hb = nc.dram_tensor("hb", [n, n_in], bf16, kind="ExternalInput")
    wt = nc.dram_tensor("wt", [n_in, d], bf16, kind="ExternalInput")
    a12 = nc.dram_tensor("a12", [d, 2], bf16, kind="ExternalInput")
    adjt = nc.dram_tensor("adjt", [n, n], mybir.dt.float8e5,
                          kind="ExternalInput")
    # out rows: [i, 0:128] = unnormalized out^T', [i, 128] = 4*r'_i
    outR = nc.dram_tensor("outR", [n, d + 1], f32, kind="ExternalOutput")

    with tile.TileContext(nc) as tc:
        with tc.tile_pool(name="const", bufs=1) as const:
            # --- constants and persistent tiles ---
            wt_sb = const.tile([128, n_kt, d], bf16)
            nc.sync.dma_start(
                out=wt_sb, in_=wt[:].rearrange("(k p) d -> p k d", p=128)
            )
            a12_sb = const.tile([d, 2], bf16)
            nc.sync.dma_start(out=a12_sb, in_=a12[:])
            ones1 = const.tile([1, 128], f32)
            nc.vector.memset(ones1, 1.0)

            # hT[k] = h[:, k*128:(k+1)*128].T  via DMA xbar transpose
            hT = const.tile([128, n_kt, n], bf16)
            for k in range(n_kt):
                nc.sync.dma_start(
                    out=hT[:, k, :],
                    in_=hb[:, k * 128 : (k + 1) * 128],
                    transpose=True,
                )

            # --- wh_ext tiles [n-tile, 129] (bf16): [Wh | 4.0] ---
            wh_ext = const.tile([128, n_jt, d + 1], bf16)
            nc.vector.memset(wh_ext, float(H))  # col d = 4.0 (head mean)
            with tc.tile_pool(name="ps_wh", bufs=2, space="PSUM") as ps_wh:
                for g in range(n_jt // 4):
                    wh_ps = ps_wh.tile([128, 4, d], f32, tag="wh_ps")
                    for q in range(4):
                        nt = g * 4 + q
                        for k in range(n_kt):
                            nc.tensor.matmul(
                                wh_ps[:, q, :],
                                hT[:, k, nt * 128 : (nt + 1) * 128],
                                wt_sb[:, k, :],
                                start=(k == 0),
                                stop=(k == n_kt - 1),
                            )
                    for q in range(4):
                        nc.vector.tensor_copy(
                            wh_ext[:, g * 4 + q, 0:d], wh_ps[:, q, :]
                        )

            # --- WhT [d, n] (bf16) ---
            whT_sb = const.tile([128, n], bf16)
            with tc.tile_pool(name="ps_whT", bufs=1, space="PSUM") as ps_whT:
                whT_ps = ps_whT.tile([128, n], f32)
                for c in range(n // 512):
                    for k in range(n_kt):
                        nc.tensor.matmul(
                            whT_ps[:, c * 512 : (c + 1) * 512],
                            wt_sb[:, k, :],
                            hT[:, k, c * 512 : (c + 1) * 512],
                            start=(k == 0),
                            stop=(k == n_kt - 1),
                        )
                nc.vector.tensor_copy(whT_sb, whT_ps)

            # --- si/sj row vectors [2, n] f32 ---
            s_sb = const.tile([2, n], f32)
            with tc.tile_pool(name="ps_s", bufs=1, space="PSUM") as ps_s:
                s_ps = ps_s.tile([2, n], f32)
                for c in range(n // 512):
                    nc.tensor.matmul(
                        s_ps[:, c * 512 : (c + 1) * 512],
                        a12_sb,
                        whT_sb[:, c * 512 : (c + 1) * 512],
                        start=True,
                        stop=True,
                    )
                nc.vector.tensor_copy(s_sb, s_ps)

            # --- Hi broadcast tile [128, n] bf16: exp(-0.8 si) ---
            Hi_bc = const.tile([128, n], bf16)
            with tc.tile_pool(name="ps_sib", bufs=1, space="PSUM") as ps_sib:
                sib_ps = ps_sib.tile([128, n], f32)
                for c in range(n // 512):
                    nc.tensor.matmul(
                        sib_ps[:, c * 512 : (c + 1) * 512],
                        ones1,
                        s_sb[0:1, c * 512 : (c + 1) * 512],
                        start=True,
                        stop=True,
                    )
                nc.scalar.activation(Hi_bc, sib_ps, AF.Exp, scale=-0.8)

            # --- Ej/Fj per-partition columns [128, n_jt] f32 ---
            Ej_cols = const.tile([128, n_jt], f32)
            Fj_cols = const.tile([128, n_jt], f32)
            with tc.tile_pool(name="ps_sj", bufs=1, space="PSUM") as ps_sj:
                sj_ps = ps_sj.tile([128, n_jt], f32)
                for t in range(n_jt):
                    nc.tensor.matmul(
                        sj_ps[:, t : t + 1],
                        whT_sb[:, t * 128 : (t + 1) * 128],
                        a12_sb[:, 1:2],
                        start=True,
                        stop=True,
                    )
                nc.scalar.activation(Ej_cols, sj_ps, AF.Exp)
                nc.scalar.activation(Fj_cols, sj_ps, AF.Exp, scale=NEG_SLOPE)

            # --- main attention loop ---
            with (
                tc.tile_pool(name="work", bufs=8) as work,
                tc.tile_pool(name="fin", bufs=2) as fin,
                tc.tile_pool(name="ps_main", bufs=1, space="PSUM") as ps_main,
            ):
                for ih in [x for x in range(ih_n)] * repeat:
                    i0 = ih * iw
                    # 16 psum blocks of [128 i, 129(pad 256)] f32, two per
                    # 2KB bank.  start=True clears has_written for the WHOLE
                    # bank, so only the even block of each bank pair issues
                    # it; the odd block's first matmul relies on
                    # "overwrite where has_written is unset" (its bits were
                    # cleared by the even neighbor's start, which the issue
                    # order guarantees happens first).
                    out_ps = ps_main.tile([128, n_ib, 256], f32, tag="out_ps")
                    for jt in range(n_jt):
                        m = work.tile([128, iw], bf16, tag="m")
                        if "dve" not in skip:
                            # P'/relu pre-mask: max(Hi_i * Fj_j, Ej_j)
                            if score_2op:
                                nc.vector.tensor_scalar(
                                    m,
                                    Hi_bc[:, i0 : i0 + iw],
                                    Fj_cols[:, jt : jt + 1],
                                    Ej_cols[:, jt : jt + 1],
                                    ALU.mult,
                                    ALU.max,
                                )
                            else:
                                t = work.tile([128, iw], bf16, tag="t")
                                nc.vector.tensor_scalar_mul(
                                    t,
                                    Hi_bc[:, i0 : i0 + iw],
                                    Fj_cols[:, jt : jt + 1],
                                )
                                nc.vector.tensor_scalar(
                                    m,
                                    t,
                                    Ej_cols[:, jt : jt + 1],
                                    None,
                                    ALU.max,
                                )
                        else:
                            nc.vector.memset(m, 1.0)
                        if "dma" not in skip:
                            # fold adjacency mask in during the load:
                            #   m += adjT' ({0, -57344}), then P' = relu(m).
                            nc.gpsimd.dma_start(
                                out=m,
                                in_=adjt[jt * 128 : (jt + 1) * 128,
                                         i0 : i0 + iw],
                                accum_op=ALU.add,
                            )
                        if "relu" not in skip:
                            p = work.tile([128, iw], bf16, tag="p")
                            if relu_act_every and jt % relu_act_every == 0:
                                nc.scalar.activation(p, m, AF.Relu)
                            else:
                                nc.vector.tensor_scalar_max(p, m, 0.0)
                        else:
                            p = m
                        if "mm" in skip:
                            continue
                        for bi in range(n_ib):
                            nc.tensor.matmul(
                                out_ps[:, bi, 0 : d + 1],
                                p[:, bi * 128 : (bi + 1) * 128],
                                wh_ext[:, jt, :],
                                start=(jt == 0 and bi % 2 == 0),
                                stop=(jt == n_jt - 1),
                                skip_group_check=(bi % 2 == 1),
                            )
                    # drain: one ACT copy for all 16 blocks, then DMA out
                    out_sb = fin.tile([128, n_ib, d + 1], f32, tag="out_sb")
                    if "mm" in skip:
                        nc.vector.memset(out_sb, 1.0)
                    else:
                        nc.scalar.activation(
                            out_sb, out_ps[:, :, 0 : d + 1], AF.Copy
                        )
                    nc.sync.dma_start(
                        out=outR[i0 : i0 + iw, :].rearrange(
                            "(b p) c -> p b c", p=128
                        ),
                        in_=out_sb,
                    )

    nc.compile()
    return nc


def _prep_inputs(h, adj, W, a):
    """Host-side shard/layout prep. Returns list of 8 per-core input dicts."""
    h_bf = np.asarray(h).astype(BF16)
    adjt_big = np.where(np.asarray(adj).T != 0, 0.0, MASK_VAL).astype(
        ml_dtypes.float8_e5m2
    )
    adjt_big = np.ascontiguousarray(adjt_big)
    W = np.asarray(W)
    a = np.asarray(a)
    in_maps = []
    for c in range(N_CORES):
        hd, b = divmod(c, B)
        wt = np.ascontiguousarray(W[hd].T).astype(BF16)          # [IN, D]
        a12 = np.stack([a[hd, :D], a[hd, D:]], axis=1).astype(BF16)  # [D, 2]
        in_maps.append(
            {"hb": np.ascontiguousarray(h_bf[b]), "wt": wt, "a12": a12,
             "adjt": adjt_big}
        )
    return in_maps


def kernel(h, adj, W, a):
    from concourse.bass_utils import run_bass_kernel_spmd

    if "nc" not in _cache:
        _cache["nc"] = _build()
    nc = _cache["nc"]

    in_maps = _prep_inputs(h, adj, W, a)
    res = run_bass_kernel_spmd(nc, in_maps, core_ids=list(range(N_CORES)))
    outs = [r["outR"] for r in res.results]  # each [N, D+1] f32

    out = np.zeros((B, N, D), dtype=np.float32)
    for c in range(N_CORES):
        hd, b = divmod(c, B)
        o = outs[c]
        r = o[:, D:]
        out[b] += np.divide(o[:, :D], r, out=np.zeros((N, D), np.float32),
                            where=r != 0)
    return out
